# revision 8
# baseline (speedup 1.0000x reference)
"""Multi-head attention (B=4, L=2048, D=768, H=12) on 8 Trainium2 NeuronCores.

Sharding: (batch, head-group). Core c handles batch c//2 and heads
6*(c%2) .. 6*(c%2)+6; it computes its 6 heads' attention output and the
partial output projection y_part = AO @ Wo[rows]; the host sums the two
partials per batch and adds bo. No collectives.

v2 pipeline (bf16 matmuls everywhere, fp32 PSUM accumulation):
  A. QT = (Wq_s^T @ xT + bq)*log2e/8  [384, 2048] bf16 (transposed layout)
     KT =  Wk_s^T @ xT + bk           [384, 2048] bf16
     V  =  x @ Wv_s + bv              [2048, 6, 65] bf16 (ones column)
  B. per head: S^T[lk, lq] = K_h @ Q_h^T chunkwise (psum = log2e * s), then
     P = 2^psum computed in bf16 on a static ACT/DVE/Pool schedule:
     ACT uses exp(psum*ln2); DVE/Pool use tensor_tensor(pow, 2.0, psum)
     (softmax is shift/scale invariant so no max subtraction; scores ~
     N(0,1) cannot overflow).  PV runs transposed: out[q, d] with
     lhsT = P-slices [128k, 128q], rhs = V [128k, 65] -- half the PE rows
     of the untransposed form.  The ones column makes psum[:, :, 64] the
     softmax denominator, indexed by q on PARTITIONS, so normalization is
     a per-partition reciprocal broadcast along free (no DMA roundtrip).
  C. AO^T tiles are transposed back via PE-transpose (identity matmul,
     bf16) into [hd, l] layout, then y_part = AO @ Wo in bf16.
"""

import numpy as np
import ml_dtypes

import concourse.mybir as mybir
import concourse.bass as bass
import concourse.tile as tile
from concourse import bacc
from concourse.bass_utils import run_bass_kernel_spmd

F32 = mybir.dt.float32
BF16 = mybir.dt.bfloat16
NPBF = ml_dtypes.bfloat16
P = 128
B, L, D, H = 4, 2048, 768, 12
HD = 64                    # head dim
HL = H // 2                # heads per core = 6
HO = HL * HD               # local output dim = 384
KC = D // P                # contraction chunks over D = 6
LC = L // P                # L chunks = 16
MC = HO // P               # output chunks for QT/KT = 3
LOG2E = 1.4426950408889634
QSCALE = LOG2E / 8.0
LN2 = 0.6931471805599453

# exp path per (head*16+lk) mod 5: 'a' = ACT exp directly from PSUM;
# 'd' = DVE copies PSUM->SBUF staging and Pool computes pow(2, .) from
# SBUF (pow only exists on GPSIMD, and GPSIMD cannot read PSUM).
LAG = 12  # subs between a scores matmul and its PV consumers
EARLY_LK = 12  # head-0 lk chunks whose scores+exp run during phase A

_NC = None


def s512(i):
    return slice(i * 512, (i + 1) * 512)


def bcast_free(ap, n):
    """Broadcast an AP along a new trailing 0-stride free dim of size n."""
    return bass.AP(ap.tensor, ap.offset, ap.ap + [[0, n]])


def build():
    nc = bacc.Bacc("TRN2", target_bir_lowering=False, debug=False)

    xT = nc.dram_tensor("xT", [D, L], BF16, kind="ExternalInput")
    wq = nc.dram_tensor("wq", [D, HO], BF16, kind="ExternalInput")
    wk = nc.dram_tensor("wk", [D, HO], BF16, kind="ExternalInput")
    wv = nc.dram_tensor("wv", [D, HO], BF16, kind="ExternalInput")
    wo = nc.dram_tensor("wo", [HO, D], BF16, kind="ExternalInput")
    bq = nc.dram_tensor("bq", [HO], F32, kind="ExternalInput")
    bk = nc.dram_tensor("bk", [HO], F32, kind="ExternalInput")
    bv = nc.dram_tensor("bv", [HO], F32, kind="ExternalInput")
    ident = nc.dram_tensor("ident", [P, P], BF16, kind="ExternalInput")
    y = nc.dram_tensor("y", [L, D], BF16, kind="ExternalOutput")

    with tile.TileContext(nc) as tc:
        with tc.tile_pool(name="static", bufs=1) as static:
            qT_tiles = [
                static.tile([P, L], BF16, name=f"qT{m}") for m in range(MC)
            ]
            kT_tiles = [
                static.tile([P, L], BF16, name=f"kT{m}") for m in range(MC)
            ]
            v_sb = static.tile([P, LC, HL, HD + 1], BF16)
            wo_sb = static.tile([P, MC, D], BF16)
            two_sb = static.tile([P, 512], F32)
            ident_sb = static.tile([P, P], BF16)
            bq_sb = static.tile([P, MC], F32)
            bk_sb = static.tile([P, MC], F32)
            bv_sb = static.tile([P, HO], F32)
            aot_tiles = [
                static.tile([P, LC, 2, HD], BF16, name=f"aot{c}")
                for c in range(MC)
            ]
            ao_tiles = [
                static.tile([P, LC, P], BF16, name=f"ao{c}") for c in range(MC)
            ]
            dummy_sb = static.tile([P, 1], F32)

            nc.vector.memset(two_sb[:], 2.0)
            nc.vector.memset(v_sb[:, :, :, HD], 1.0)
            # preload the exp activation table while DMAs run
            nc.scalar.activation(
                out=dummy_sb[:],
                in_=two_sb[:, 0:1],
                func=mybir.ActivationFunctionType.Exp,
            )

            p_ts = {}

            def emit_exp(out_ap, s_t, eng):
                if eng == "a":
                    nc.scalar.activation(
                        out=out_ap,
                        in_=s_t[:, :],
                        func=mybir.ActivationFunctionType.Exp,
                        scale=LN2,
                    )
                elif eng == "s":
                    nc.vector.tensor_scalar(
                        out_ap.bitcast(mybir.dt.uint16),
                        s_t[:, :],
                        128.0,
                        16251.0,
                        mybir.AluOpType.mult,
                        mybir.AluOpType.add,
                    )
                else:
                    stage = stpool.tile([P, 512], F32, tag="st")
                    nc.vector.tensor_copy(stage[:, :], s_t[:, :])
                    nc.gpsimd.tensor_tensor(
                        out_ap, two_sb[:, :], stage[:, :], mybir.AluOpType.pow
                    )

            def scores_mm(hl, lk, j, s_t):
                pc, odd = hl // 2, hl % 2
                r0 = odd * HD
                nc.tensor.matmul(
                    s_t[:, :],
                    kT_tiles[pc][r0 : r0 + HD, lk * P : (lk + 1) * P],
                    qT_tiles[pc][r0 : r0 + HD, s512(j)],
                    start=True,
                    stop=True,
                )

            # ---------------- phase A: projections ----------------
            with (
                tc.tile_pool(name="xpool", bufs=1) as xpool,
                tc.tile_pool(name="wpool", bufs=2) as wpool,
            ):
                # DMA in first-use order: QK job 0 needs (wq c, xc c) pairs,
                # V job 0 then needs wv; K jobs follow at i=6.
                wv_sb = wpool.tile([P, KC, HO], BF16, tag="w")
                w_tiles = {
                    "q": wpool.tile([P, KC, HO], BF16, tag="w", name="wq_sb"),
                    "k": wpool.tile([P, KC, HO], BF16, tag="w", name="wk_sb"),
                }
                xT_chunks = [
                    xpool.tile([P, L], BF16, name=f"xc{c}") for c in range(KC)
                ]
                # x arrives split by L-half: the h0 QK jobs need only
                # half of x, so PE turns compute-bound ~1us in.
                for c in range(KC):
                    nc.sync.dma_start(
                        w_tiles["q"][:, c, :], wq[c * P : (c + 1) * P, :]
                    )
                    xeng = nc.sync if c % 2 == 0 else nc.scalar
                    xeng.dma_start(
                        xT_chunks[c][:, 0:1024], xT[c * P : (c + 1) * P, 0:1024]
                    )
                    if c == 0:
                        nc.scalar.dma_start(
                            bq_sb[:], bq.ap().rearrange("(c p) -> p c", p=P)
                        )
                for c in range(KC):
                    nc.sync.dma_start(
                        w_tiles["k"][:, c, :], wk[c * P : (c + 1) * P, :]
                    )
                    xeng = nc.sync if c % 2 == 1 else nc.scalar
                    xeng.dma_start(
                        xT_chunks[c][:, 1024:L], xT[c * P : (c + 1) * P, 1024:L]
                    )
                    if c == 0:
                        nc.scalar.dma_start(
                            bk_sb[:], bk.ap().rearrange("(c p) -> p c", p=P)
                        )
                for c in range(KC):
                    nc.scalar.dma_start(wv_sb[:, c, :], wv[c * P : (c + 1) * P, :])
                nc.scalar.dma_start(bv_sb[:], bv[None, :].partition_broadcast(P))
                nc.sync.dma_start(ident_sb[:, :], ident[:, :])
                for c in range(MC):
                    nc.scalar.dma_start(wo_sb[:, c, :], wo[c * P : (c + 1) * P, :])

                bv_v = bv_sb.rearrange("p (h d) -> p h d", d=HD)

                def start_qk(i, pool):
                    which, m, h = i // 6, (i // 2) % 3, i % 2
                    ps = pool.tile([P, 1024], F32, tag="pj", name=f"qk{i}")
                    return (which, m, h, ps)

                def job_mm_qk(job, k):
                    which, m, h, ps = job
                    w_sb = w_tiles["q" if which == 0 else "k"]
                    for n2 in range(2):
                        j = h * 2 + n2
                        nc.tensor.matmul(
                            ps[:, n2 * 512 : (n2 + 1) * 512],
                            w_sb[:, k, m * P : (m + 1) * P],
                            xT_chunks[k][:, s512(j)],
                            start=(k == 0),
                            stop=(k == KC - 1),
                        )

                def evict_qk(job):
                    which, m, h, ps = job
                    b_sb = bq_sb if which == 0 else bk_sb
                    out_sb = (qT_tiles if which == 0 else kT_tiles)[m]
                    out_ap = out_sb[:, h * 1024 : (h + 1) * 1024]
                    nc.vector.tensor_scalar(
                        out_ap,
                        ps[:, :],
                        b_sb[:, m : m + 1],
                        QSCALE if which == 0 else 1.0,
                        mybir.AluOpType.add,
                        mybir.AluOpType.mult,
                    )

                def start_v(l):
                    ps = vps.tile([P, HO], F32, tag="v", name=f"vj{l}")
                    return (l, ps)

                def job_mm_v(job, k):
                    l, ps = job
                    nc.tensor.matmul(
                        ps[:, :],
                        xT_chunks[k][:, l * P : (l + 1) * P],
                        wv_sb[:, k, :],
                        start=(k == 0),
                        stop=(k == KC - 1),
                    )

                def evict_v(job):
                    l, ps = job
                    nc.vector.tensor_tensor(
                        v_sb[:, l, :, 0:HD],
                        ps[:, :].rearrange("p (h d) -> p h d", d=HD),
                        bv_v,
                        mybir.AluOpType.add,
                    )

                with (
                    tc.tile_pool(name="pj3", bufs=3, space="PSUM") as pj3,
                    tc.tile_pool(name="vps", bufs=2, space="PSUM") as vps,
                ):
                    # job groups by L-half: [q-h0, k-h0, q-h1, k-h1], each
                    # 3 jobs k-outer (6 banks) so every arriving chunk
                    # feeds 1.28us of PE against ~1us of DMA.
                    def run_v(l):
                        jv = start_v(l)
                        for k in range(KC):
                            job_mm_v(jv, k)
                        evict_v(jv)

                    for gi, grp in enumerate(
                        ([0, 2, 4], [6, 8, 10], [1, 3, 5], [7, 9, 11])
                    ):
                        jobs = [start_qk(i, pj3) for i in grp]
                        for k in range(KC):
                            for job in jobs:
                                job_mm_qk(job, k)
                        for job in jobs:
                            evict_qk(job)
                        if gi == 3:
                            for l in range(4):
                                run_v(l)
                    for l in range(4, 16):
                        run_v(l)

            # ---------------- phase B: attention per head ----------------
            with (
                tc.tile_pool(name="ppool", bufs=28) as ppool,
                tc.tile_pool(name="stpool", bufs=8) as stpool,
                tc.tile_pool(name="dpool", bufs=4) as dpool,
                tc.tile_pool(name="sps", bufs=5, space="PSUM") as sps,
                tc.tile_pool(name="ops", bufs=3, space="PSUM") as ops,
            ):
                # Stream of 512-column subs (hl, lk, j): scores matmul ->
                # exp op.  A full head's P tiles stay resident; its PV runs
                # as 16 consecutive 16-matmul qc-chains interleaved into the
                # NEXT head's sub stream.  Chains pack 7+7+2 per PSUM bank
                # (one open accumulation group per 2KB zero region at a
                # time; a chain's [128, 65] output may not cross a bank).
                QCG = (7, 7, 2)

                def emit_scores_exp(hl, lk, j):
                    if j == 0:
                        p_ts[(hl, lk)] = ppool.tile(
                            [P, L], BF16, tag="p", name=f"p{hl}_{lk}"
                        )
                    idx = hl * LC + lk
                    if idx in (17, 33, 49, 65, 81, 3, 11, 27, 59, 75, 5, 21, 37, 69):
                        eng = "s"
                    else:
                        eng = "d" if lk < 14 and idx % 2 == 1 else "a"
                    s_t = sps.tile([P, 512], F32, tag="s")
                    scores_mm(hl, lk, j, s_t)
                    emit_exp(p_ts[(hl, lk)][:, j * 512 : (j + 1) * 512], s_t, eng)

                def emit_chain(hl, qc, psum_o):
                    g = 0 if qc < 7 else (1 if qc < 14 else 2)
                    slot = qc - (0, 7, 14)[g]
                    for lk in range(LC):
                        nc.tensor.matmul(
                            psum_o[g][:, slot, :],
                            p_ts[(hl, lk)][:, qc * P : (qc + 1) * P],
                            v_sb[:, lk, hl, :],
                            start=(lk == 0),
                            stop=(lk == LC - 1),
                        )
                    if qc == LC - 1:
                        for lk in range(LC):
                            del p_ts[(hl, lk)]

                def emit_evict(hl, psum_o):
                    pc, odd = hl // 2, hl % 2
                    den = dpool.tile([P, LC], F32, tag="den")
                    rec = dpool.tile([P, LC], F32, tag="rec")
                    q0 = 0
                    for g, n in enumerate(QCG):
                        nc.vector.tensor_copy(
                            den[:, q0 : q0 + n], psum_o[g][:, :, HD]
                        )
                        q0 += n
                    nc.vector.reciprocal(rec[:, :], den[:, :])
                    q0 = 0
                    for g, n in enumerate(QCG):
                        nc.vector.tensor_tensor(
                            aot_tiles[pc][:, q0 : q0 + n, odd, :],
                            psum_o[g][:, :, 0:HD],
                            bcast_free(rec[:, q0 : q0 + n], HD),
                            mybir.AluOpType.mult,
                        )
                        q0 += n

                def alloc_psum(hl):
                    return [
                        ops.tile([P, n, HD + 1], F32, tag="o", name=f"o{hl}_{g}")
                        for g, n in enumerate(QCG)
                    ]

                pend = None  # (head, psum_o) whose PV chains interleave now
                for hl in range(HL):
                    nchain = 0
                    for lk in range(LC):
                        for j in range(4):
                            emit_scores_exp(hl, lk, j)
                            sub = lk * 4 + j
                            if pend is not None and sub >= 3 and sub % 4 == 3:
                                if nchain < LC:
                                    emit_chain(pend[0], nchain, pend[1])
                                    nchain += 1
                                    if nchain == LC:
                                        emit_evict(pend[0], pend[1])
                                        pend = None
                    if pend is not None:
                        for qc in range(nchain, LC):
                            emit_chain(pend[0], qc, pend[1])
                        emit_evict(pend[0], pend[1])
                    pend = (hl, alloc_psum(hl))
                for qc in range(LC):
                    emit_chain(pend[0], qc, pend[1])
                    if qc in (3, 11):
                        # pc0/pc1 AO transposes ride the freed scores-psum
                        # slots during the tail chains; phase C then only
                        # has pc2 on its critical path
                        pc = 0 if qc == 3 else 1
                        for hb in range(2):
                            t_ps = sps.tile(
                                [P, 8, P], BF16, tag="s", name=f"tp{pc}_{hb}"
                            )
                            for q2 in range(8):
                                nc.tensor.transpose(
                                    t_ps[:, q2, :],
                                    aot_tiles[pc][
                                        :, hb * 8 + q2, :, :
                                    ].rearrange("p a b -> p (a b)"),
                                    ident_sb[:, :],
                                )
                            nc.vector.tensor_copy(
                                ao_tiles[pc][:, hb * 8 : hb * 8 + 8, :],
                                t_ps[:, :, :],
                            )
                emit_evict(pend[0], pend[1])

            # ---------------- phase C: transpose + output projection ----
            with (
                tc.tile_pool(name="ypool", bufs=6) as ypool,
                tc.tile_pool(name="tps", bufs=2, space="PSUM") as tpsp,
                tc.tile_pool(name="yps", bufs=3, space="PSUM") as yps,
            ):
                for hb in range(2):
                    t_ps = tpsp.tile([P, 8, P], BF16, tag="t")
                    for q2 in range(8):
                        qc = hb * 8 + q2
                        nc.tensor.transpose(
                            t_ps[:, q2, :],
                            aot_tiles[2][:, qc, :, :].rearrange(
                                "p a b -> p (a b)"
                            ),
                            ident_sb[:, :],
                        )
                    if hb == 0:
                        nc.vector.tensor_copy(
                            ao_tiles[2][:, 0:8, :], t_ps[:, :, :]
                        )
                    else:
                        nc.scalar.activation(
                            out=ao_tiles[2][:, 8:LC, :],
                            in_=t_ps[:, :, :],
                            func=mybir.ActivationFunctionType.Copy,
                        )

                for m in range(LC):
                    ps = yps.tile([P, D], F32, tag="y")
                    for c in range(MC):
                        for n0, nsz in ((0, 512), (512, 256)):
                            nc.tensor.matmul(
                                ps[:, n0 : n0 + nsz],
                                ao_tiles[c][:, m, :],
                                wo_sb[:, c, n0 : n0 + nsz],
                                start=(c == 0),
                                stop=(c == MC - 1),
                            )
                    y_t = ypool.tile([P, D], BF16, tag="yt")
                    if m % 2 == 0:
                        nc.vector.tensor_copy(y_t[:], ps[:])
                    else:
                        nc.scalar.activation(
                            out=y_t[:],
                            in_=ps[:],
                            func=mybir.ActivationFunctionType.Copy,
                        )
                    yeng = nc.sync if m % 2 == 0 else nc.scalar
                    yeng.dma_start(y[m * P : (m + 1) * P, :], y_t[:])

    nc.compile()
    return nc


def _get_nc():
    global _NC
    if _NC is None:
        _NC = build()
    return _NC


def kernel(**inputs) -> np.ndarray:
    x = np.asarray(inputs["x"], dtype=np.float32)
    Wq = np.asarray(inputs["Wq"], dtype=np.float32)
    Wk = np.asarray(inputs["Wk"], dtype=np.float32)
    Wv = np.asarray(inputs["Wv"], dtype=np.float32)
    Wo = np.asarray(inputs["Wo"], dtype=np.float32)
    bq = np.asarray(inputs["bq"], dtype=np.float32)
    bk = np.asarray(inputs["bk"], dtype=np.float32)
    bv = np.asarray(inputs["bv"], dtype=np.float32)
    bo = np.asarray(inputs["bo"], dtype=np.float32)

    nc = _get_nc()
    ident = np.eye(P, dtype=NPBF)

    in_maps = []
    for c in range(8):
        b, hg = c // 2, c % 2
        cs = slice(hg * HO, (hg + 1) * HO)
        in_maps.append(
            {
                "xT": np.ascontiguousarray(x[b].T).astype(NPBF),
                "wq": np.ascontiguousarray(Wq[:, cs]).astype(NPBF),
                "wk": np.ascontiguousarray(Wk[:, cs]).astype(NPBF),
                "wv": np.ascontiguousarray(Wv[:, cs]).astype(NPBF),
                "wo": np.ascontiguousarray(Wo[cs, :]).astype(NPBF),
                "bq": np.ascontiguousarray(bq[cs]),
                "bk": np.ascontiguousarray(bk[cs]),
                "bv": np.ascontiguousarray(bv[cs]),
                "ident": ident,
            }
        )

    res = run_bass_kernel_spmd(nc, in_maps, core_ids=list(range(8)))
    out = np.empty((B, L, D), dtype=np.float32)
    for b in range(B):
        out[b] = (
            res.results[2 * b]["y"].astype(np.float32)
            + res.results[2 * b + 1]["y"].astype(np.float32)
            + bo
        )
    return out


# revision 9
# speedup vs baseline: 1.0020x; 1.0020x over previous
"""Multi-head attention (B=4, L=2048, D=768, H=12) on 8 Trainium2 NeuronCores.

Sharding: (batch, head-group). Core c handles batch c//2 and heads
6*(c%2) .. 6*(c%2)+6; it computes its 6 heads' attention output and the
partial output projection y_part = AO @ Wo[rows]; the host sums the two
partials per batch and adds bo. No collectives.

v2 pipeline (bf16 matmuls everywhere, fp32 PSUM accumulation):
  A. QT = (Wq_s^T @ xT + bq)*log2e/8  [384, 2048] bf16 (transposed layout)
     KT =  Wk_s^T @ xT + bk           [384, 2048] bf16
     V  =  x @ Wv_s + bv              [2048, 6, 65] bf16 (ones column)
  B. per head: S^T[lk, lq] = K_h @ Q_h^T chunkwise (psum = log2e * s), then
     P = 2^psum computed in bf16 on a static ACT/DVE/Pool schedule:
     ACT uses exp(psum*ln2); DVE/Pool use tensor_tensor(pow, 2.0, psum)
     (softmax is shift/scale invariant so no max subtraction; scores ~
     N(0,1) cannot overflow).  PV runs transposed: out[q, d] with
     lhsT = P-slices [128k, 128q], rhs = V [128k, 65] -- half the PE rows
     of the untransposed form.  The ones column makes psum[:, :, 64] the
     softmax denominator, indexed by q on PARTITIONS, so normalization is
     a per-partition reciprocal broadcast along free (no DMA roundtrip).
  C. AO^T tiles are transposed back via PE-transpose (identity matmul,
     bf16) into [hd, l] layout, then y_part = AO @ Wo in bf16.
"""

import numpy as np
import ml_dtypes

import concourse.mybir as mybir
import concourse.bass as bass
import concourse.tile as tile
from concourse import bacc
from concourse.bass_utils import run_bass_kernel_spmd

F32 = mybir.dt.float32
BF16 = mybir.dt.bfloat16
NPBF = ml_dtypes.bfloat16
P = 128
B, L, D, H = 4, 2048, 768, 12
HD = 64                    # head dim
HL = H // 2                # heads per core = 6
HO = HL * HD               # local output dim = 384
KC = D // P                # contraction chunks over D = 6
LC = L // P                # L chunks = 16
MC = HO // P               # output chunks for QT/KT = 3
LOG2E = 1.4426950408889634
QSCALE = LOG2E / 8.0
LN2 = 0.6931471805599453

# exp path per (head*16+lk) mod 5: 'a' = ACT exp directly from PSUM;
# 'd' = DVE copies PSUM->SBUF staging and Pool computes pow(2, .) from
# SBUF (pow only exists on GPSIMD, and GPSIMD cannot read PSUM).
LAG = 12  # subs between a scores matmul and its PV consumers
EARLY_LK = 12  # head-0 lk chunks whose scores+exp run during phase A

_NC = None


def s512(i):
    return slice(i * 512, (i + 1) * 512)


def bcast_free(ap, n):
    """Broadcast an AP along a new trailing 0-stride free dim of size n."""
    return bass.AP(ap.tensor, ap.offset, ap.ap + [[0, n]])


def build():
    nc = bacc.Bacc("TRN2", target_bir_lowering=False, debug=False)

    xT = nc.dram_tensor("xT", [D, L], BF16, kind="ExternalInput")
    wq = nc.dram_tensor("wq", [D, HO], BF16, kind="ExternalInput")
    wk = nc.dram_tensor("wk", [D, HO], BF16, kind="ExternalInput")
    wv = nc.dram_tensor("wv", [D, HO], BF16, kind="ExternalInput")
    wo = nc.dram_tensor("wo", [HO, D], BF16, kind="ExternalInput")
    bq = nc.dram_tensor("bq", [HO], F32, kind="ExternalInput")
    bk = nc.dram_tensor("bk", [HO], F32, kind="ExternalInput")
    bv = nc.dram_tensor("bv", [HO], F32, kind="ExternalInput")
    ident = nc.dram_tensor("ident", [P, P], BF16, kind="ExternalInput")
    y = nc.dram_tensor("y", [L, D], BF16, kind="ExternalOutput")

    with tile.TileContext(nc) as tc:
        with tc.tile_pool(name="static", bufs=1) as static:
            qT_tiles = [
                static.tile([P, L], BF16, name=f"qT{m}") for m in range(MC)
            ]
            kT_tiles = [
                static.tile([P, L], BF16, name=f"kT{m}") for m in range(MC)
            ]
            v_sb = static.tile([P, LC, HL, HD + 1], BF16)
            wo_sb = static.tile([P, MC, D], BF16)
            two_sb = static.tile([P, 512], F32)
            ident_sb = static.tile([P, P], BF16)
            bq_sb = static.tile([P, MC], F32)
            bk_sb = static.tile([P, MC], F32)
            bv_sb = static.tile([P, HO], F32)
            aot_tiles = [
                static.tile([P, LC, 2, HD], BF16, name=f"aot{c}")
                for c in range(MC)
            ]
            ao_tiles = [
                static.tile([P, LC, P], BF16, name=f"ao{c}") for c in range(MC)
            ]
            dummy_sb = static.tile([P, 1], F32)

            nc.vector.memset(two_sb[:], 2.0)
            nc.vector.memset(v_sb[:, :, :, HD], 1.0)
            # preload the exp activation table while DMAs run
            nc.scalar.activation(
                out=dummy_sb[:],
                in_=two_sb[:, 0:1],
                func=mybir.ActivationFunctionType.Exp,
            )

            p_ts = {}

            def emit_exp(out_ap, s_t, eng):
                if eng == "a":
                    nc.scalar.activation(
                        out=out_ap,
                        in_=s_t[:, :],
                        func=mybir.ActivationFunctionType.Exp,
                        scale=LN2,
                    )
                elif eng == "s":
                    nc.vector.tensor_scalar(
                        out_ap.bitcast(mybir.dt.uint16),
                        s_t[:, :],
                        128.0,
                        16251.0,
                        mybir.AluOpType.mult,
                        mybir.AluOpType.add,
                    )
                else:
                    stage = stpool.tile([P, 512], F32, tag="st")
                    nc.vector.tensor_copy(stage[:, :], s_t[:, :])
                    nc.gpsimd.tensor_tensor(
                        out_ap, two_sb[:, :], stage[:, :], mybir.AluOpType.pow
                    )

            def scores_mm(hl, lk, j, s_t):
                pc, odd = hl // 2, hl % 2
                r0 = odd * HD
                nc.tensor.matmul(
                    s_t[:, :],
                    kT_tiles[pc][r0 : r0 + HD, lk * P : (lk + 1) * P],
                    qT_tiles[pc][r0 : r0 + HD, s512(j)],
                    start=True,
                    stop=True,
                )

            # ---------------- phase A: projections ----------------
            with (
                tc.tile_pool(name="xpool", bufs=1) as xpool,
                tc.tile_pool(name="wpool", bufs=2) as wpool,
            ):
                # DMA in first-use order: QK job 0 needs (wq c, xc c) pairs,
                # V job 0 then needs wv; K jobs follow at i=6.
                wv_sb = wpool.tile([P, KC, HO], BF16, tag="w")
                w_tiles = {
                    "q": wpool.tile([P, KC, HO], BF16, tag="w", name="wq_sb"),
                    "k": wpool.tile([P, KC, HO], BF16, tag="w", name="wk_sb"),
                }
                xT_chunks = [
                    xpool.tile([P, L], BF16, name=f"xc{c}") for c in range(KC)
                ]
                # x arrives split by L-half: the h0 QK jobs need only
                # half of x, so PE turns compute-bound ~1us in.
                for c in range(KC):
                    nc.sync.dma_start(
                        w_tiles["q"][:, c, :], wq[c * P : (c + 1) * P, :]
                    )
                    xeng = nc.sync if c % 2 == 0 else nc.scalar
                    xeng.dma_start(
                        xT_chunks[c][:, 0:1024], xT[c * P : (c + 1) * P, 0:1024]
                    )
                    if c == 0:
                        nc.scalar.dma_start(
                            bq_sb[:], bq.ap().rearrange("(c p) -> p c", p=P)
                        )
                for c in range(KC):
                    nc.sync.dma_start(
                        w_tiles["k"][:, c, :], wk[c * P : (c + 1) * P, :]
                    )
                    xeng = nc.sync if c % 2 == 1 else nc.scalar
                    xeng.dma_start(
                        xT_chunks[c][:, 1024:L], xT[c * P : (c + 1) * P, 1024:L]
                    )
                    if c == 0:
                        nc.scalar.dma_start(
                            bk_sb[:], bk.ap().rearrange("(c p) -> p c", p=P)
                        )
                for c in range(KC):
                    nc.scalar.dma_start(wv_sb[:, c, :], wv[c * P : (c + 1) * P, :])
                nc.scalar.dma_start(bv_sb[:], bv[None, :].partition_broadcast(P))
                nc.sync.dma_start(ident_sb[:, :], ident[:, :])
                for c in range(MC):
                    nc.scalar.dma_start(wo_sb[:, c, :], wo[c * P : (c + 1) * P, :])

                bv_v = bv_sb.rearrange("p (h d) -> p h d", d=HD)

                def start_qk(i, pool):
                    which, m, h = i // 6, (i // 2) % 3, i % 2
                    ps = pool.tile([P, 1024], F32, tag="pj", name=f"qk{i}")
                    return (which, m, h, ps)

                def job_mm_qk(job, k):
                    which, m, h, ps = job
                    w_sb = w_tiles["q" if which == 0 else "k"]
                    for n2 in range(2):
                        j = h * 2 + n2
                        nc.tensor.matmul(
                            ps[:, n2 * 512 : (n2 + 1) * 512],
                            w_sb[:, k, m * P : (m + 1) * P],
                            xT_chunks[k][:, s512(j)],
                            start=(k == 0),
                            stop=(k == KC - 1),
                        )

                def evict_qk(job):
                    which, m, h, ps = job
                    b_sb = bq_sb if which == 0 else bk_sb
                    out_sb = (qT_tiles if which == 0 else kT_tiles)[m]
                    out_ap = out_sb[:, h * 1024 : (h + 1) * 1024]
                    nc.vector.tensor_scalar(
                        out_ap,
                        ps[:, :],
                        b_sb[:, m : m + 1],
                        QSCALE if which == 0 else 1.0,
                        mybir.AluOpType.add,
                        mybir.AluOpType.mult,
                    )

                def start_v(l):
                    ps = vps.tile([P, HO], F32, tag="v", name=f"vj{l}")
                    return (l, ps)

                def job_mm_v(job, k):
                    l, ps = job
                    nc.tensor.matmul(
                        ps[:, :],
                        xT_chunks[k][:, l * P : (l + 1) * P],
                        wv_sb[:, k, :],
                        start=(k == 0),
                        stop=(k == KC - 1),
                    )

                def evict_v(job):
                    l, ps = job
                    nc.vector.tensor_tensor(
                        v_sb[:, l, :, 0:HD],
                        ps[:, :].rearrange("p (h d) -> p h d", d=HD),
                        bv_v,
                        mybir.AluOpType.add,
                    )

                with (
                    tc.tile_pool(name="pj3", bufs=3, space="PSUM") as pj3,
                    tc.tile_pool(name="vps", bufs=2, space="PSUM") as vps,
                ):
                    # job groups by L-half: [q-h0, k-h0, q-h1, k-h1], each
                    # 3 jobs k-outer (6 banks) so every arriving chunk
                    # feeds 1.28us of PE against ~1us of DMA.
                    def run_v(l):
                        jv = start_v(l)
                        for k in range(KC):
                            job_mm_v(jv, k)
                        evict_v(jv)

                    for gi, grp in enumerate(
                        ([0, 2, 4], [6, 8, 10], [1, 3, 5], [7, 9, 11])
                    ):
                        jobs = [start_qk(i, pj3) for i in grp]
                        for k in range(KC):
                            for job in jobs:
                                job_mm_qk(job, k)
                        for job in jobs:
                            evict_qk(job)
                        if gi == 3:
                            for l in range(4):
                                run_v(l)
                    for l in range(4, 16):
                        run_v(l)

            # ---------------- phase B: attention per head ----------------
            with (
                tc.tile_pool(name="ppool", bufs=28) as ppool,
                tc.tile_pool(name="stpool", bufs=8) as stpool,
                tc.tile_pool(name="dpool", bufs=4) as dpool,
                tc.tile_pool(name="sps", bufs=5, space="PSUM") as sps,
                tc.tile_pool(name="ops", bufs=3, space="PSUM") as ops,
            ):
                # Stream of 512-column subs (hl, lk, j): scores matmul ->
                # exp op.  A full head's P tiles stay resident; its PV runs
                # as 16 consecutive 16-matmul qc-chains interleaved into the
                # NEXT head's sub stream.  Chains pack 7+7+2 per PSUM bank
                # (one open accumulation group per 2KB zero region at a
                # time; a chain's [128, 65] output may not cross a bank).
                QCG = (7, 7, 2)

                def emit_scores_exp(hl, lk, j):
                    if j == 0:
                        p_ts[(hl, lk)] = ppool.tile(
                            [P, L], BF16, tag="p", name=f"p{hl}_{lk}"
                        )
                    idx = hl * LC + lk
                    if idx in (17, 33, 49, 65, 81, 3, 11, 27, 59, 75, 5, 21, 37, 69):
                        eng = "s"
                    else:
                        eng = "d" if lk < 14 and idx % 2 == 1 else "a"
                    s_t = sps.tile([P, 512], F32, tag="s")
                    scores_mm(hl, lk, j, s_t)
                    emit_exp(p_ts[(hl, lk)][:, j * 512 : (j + 1) * 512], s_t, eng)

                def emit_chain(hl, qc, psum_o):
                    g = 0 if qc < 7 else (1 if qc < 14 else 2)
                    slot = qc - (0, 7, 14)[g]
                    for lk in range(LC):
                        nc.tensor.matmul(
                            psum_o[g][:, slot, :],
                            p_ts[(hl, lk)][:, qc * P : (qc + 1) * P],
                            v_sb[:, lk, hl, :],
                            start=(lk == 0),
                            stop=(lk == LC - 1),
                        )
                    if qc == LC - 1:
                        for lk in range(LC):
                            del p_ts[(hl, lk)]

                def emit_evict(hl, psum_o):
                    pc, odd = hl // 2, hl % 2
                    den = dpool.tile([P, LC], F32, tag="den")
                    rec = dpool.tile([P, LC], F32, tag="rec")
                    q0 = 0
                    for g, n in enumerate(QCG):
                        nc.scalar.activation(
                            out=den[:, q0 : q0 + n],
                            in_=psum_o[g][:, :, HD],
                            func=mybir.ActivationFunctionType.Copy,
                        )
                        q0 += n
                    nc.vector.reciprocal(rec[:, :], den[:, :])
                    q0 = 0
                    for g, n in enumerate(QCG):
                        nc.vector.tensor_tensor(
                            aot_tiles[pc][:, q0 : q0 + n, odd, :],
                            psum_o[g][:, :, 0:HD],
                            bcast_free(rec[:, q0 : q0 + n], HD),
                            mybir.AluOpType.mult,
                        )
                        q0 += n

                def alloc_psum(hl):
                    return [
                        ops.tile([P, n, HD + 1], F32, tag="o", name=f"o{hl}_{g}")
                        for g, n in enumerate(QCG)
                    ]

                pend = None  # (head, psum_o) whose PV chains interleave now
                for hl in range(HL):
                    nchain = 0
                    for lk in range(LC):
                        for j in range(4):
                            emit_scores_exp(hl, lk, j)
                            sub = lk * 4 + j
                            if pend is not None and sub >= 3 and sub % 4 == 3:
                                if nchain < LC:
                                    emit_chain(pend[0], nchain, pend[1])
                                    nchain += 1
                                    if nchain == LC:
                                        emit_evict(pend[0], pend[1])
                                        pend = None
                    if pend is not None:
                        for qc in range(nchain, LC):
                            emit_chain(pend[0], qc, pend[1])
                        emit_evict(pend[0], pend[1])
                    pend = (hl, alloc_psum(hl))
                for qc in range(LC):
                    emit_chain(pend[0], qc, pend[1])
                    if qc in (3, 11):
                        # pc0/pc1 AO transposes ride the freed scores-psum
                        # slots during the tail chains; phase C then only
                        # has pc2 on its critical path
                        pc = 0 if qc == 3 else 1
                        for hb in range(2):
                            t_ps = sps.tile(
                                [P, 8, P], BF16, tag="s", name=f"tp{pc}_{hb}"
                            )
                            for q2 in range(8):
                                nc.tensor.transpose(
                                    t_ps[:, q2, :],
                                    aot_tiles[pc][
                                        :, hb * 8 + q2, :, :
                                    ].rearrange("p a b -> p (a b)"),
                                    ident_sb[:, :],
                                )
                            nc.vector.tensor_copy(
                                ao_tiles[pc][:, hb * 8 : hb * 8 + 8, :],
                                t_ps[:, :, :],
                            )
                emit_evict(pend[0], pend[1])

            # ---------------- phase C: transpose + output projection ----
            with (
                tc.tile_pool(name="ypool", bufs=6) as ypool,
                tc.tile_pool(name="tps", bufs=2, space="PSUM") as tpsp,
                tc.tile_pool(name="yps", bufs=3, space="PSUM") as yps,
            ):
                for hb in range(2):
                    t_ps = tpsp.tile([P, 8, P], BF16, tag="t")
                    for q2 in range(8):
                        qc = hb * 8 + q2
                        nc.tensor.transpose(
                            t_ps[:, q2, :],
                            aot_tiles[2][:, qc, :, :].rearrange(
                                "p a b -> p (a b)"
                            ),
                            ident_sb[:, :],
                        )
                    if hb == 0:
                        nc.vector.tensor_copy(
                            ao_tiles[2][:, 0:8, :], t_ps[:, :, :]
                        )
                    else:
                        nc.scalar.activation(
                            out=ao_tiles[2][:, 8:LC, :],
                            in_=t_ps[:, :, :],
                            func=mybir.ActivationFunctionType.Copy,
                        )

                for m in range(LC):
                    ps = yps.tile([P, D], F32, tag="y")
                    for c in range(MC):
                        for n0, nsz in ((0, 512), (512, 256)):
                            nc.tensor.matmul(
                                ps[:, n0 : n0 + nsz],
                                ao_tiles[c][:, m, :],
                                wo_sb[:, c, n0 : n0 + nsz],
                                start=(c == 0),
                                stop=(c == MC - 1),
                            )
                    y_t = ypool.tile([P, D], BF16, tag="yt")
                    if m % 2 == 0:
                        nc.vector.tensor_copy(y_t[:], ps[:])
                    else:
                        nc.scalar.activation(
                            out=y_t[:],
                            in_=ps[:],
                            func=mybir.ActivationFunctionType.Copy,
                        )
                    yeng = nc.sync if m % 2 == 0 else nc.scalar
                    yeng.dma_start(y[m * P : (m + 1) * P, :], y_t[:])

    nc.compile()
    return nc


def _get_nc():
    global _NC
    if _NC is None:
        _NC = build()
    return _NC


def kernel(**inputs) -> np.ndarray:
    x = np.asarray(inputs["x"], dtype=np.float32)
    Wq = np.asarray(inputs["Wq"], dtype=np.float32)
    Wk = np.asarray(inputs["Wk"], dtype=np.float32)
    Wv = np.asarray(inputs["Wv"], dtype=np.float32)
    Wo = np.asarray(inputs["Wo"], dtype=np.float32)
    bq = np.asarray(inputs["bq"], dtype=np.float32)
    bk = np.asarray(inputs["bk"], dtype=np.float32)
    bv = np.asarray(inputs["bv"], dtype=np.float32)
    bo = np.asarray(inputs["bo"], dtype=np.float32)

    nc = _get_nc()
    ident = np.eye(P, dtype=NPBF)

    in_maps = []
    for c in range(8):
        b, hg = c // 2, c % 2
        cs = slice(hg * HO, (hg + 1) * HO)
        in_maps.append(
            {
                "xT": np.ascontiguousarray(x[b].T).astype(NPBF),
                "wq": np.ascontiguousarray(Wq[:, cs]).astype(NPBF),
                "wk": np.ascontiguousarray(Wk[:, cs]).astype(NPBF),
                "wv": np.ascontiguousarray(Wv[:, cs]).astype(NPBF),
                "wo": np.ascontiguousarray(Wo[cs, :]).astype(NPBF),
                "bq": np.ascontiguousarray(bq[cs]),
                "bk": np.ascontiguousarray(bk[cs]),
                "bv": np.ascontiguousarray(bv[cs]),
                "ident": ident,
            }
        )

    res = run_bass_kernel_spmd(nc, in_maps, core_ids=list(range(8)))
    out = np.empty((B, L, D), dtype=np.float32)
    for b in range(B):
        out[b] = (
            res.results[2 * b]["y"].astype(np.float32)
            + res.results[2 * b + 1]["y"].astype(np.float32)
            + bo
        )
    return out
